# revision 12
# baseline (speedup 1.0000x reference)
"""Trainium2 Bass kernel for CPELayer_ResAG (concept-routed LoRA edit layer).

Computation (per token t with concept c = concept_idx[t]):
    down = edit_direction[t] @ lora_down[c]          # [768]@[768,4] -> [4]
    up   = down @ lora_up[c]                         # [4]@[4,1280]  -> [1280]
    out  = x[t] @ W.T + b_lin + 0.25 * up

Strategy: data-parallel over batch across 8 cores (616 tokens/core).
The routed LoRA is computed densely for ALL concepts (A.T = ld.T @ ed.T,
~6% extra PE work), masked on-device with a one-hot built by DVE is_equal,
and contracted back with lora_up, accumulating into the same PSUM group as
the org matmul.

v2 schedule (vs v1): LoRA-branch matmuls run in fp8-e4m3 with
perf_mode=DoubleRow (2 contraction rows per PE cell: operands are 3D
[128, 2, N] tiles, contraction row = subtile*128 + partition), halving
their PE column counts.  All power-of-two scale corrections (ld x16 on
the host, x1/128 in the mask value, lu x2 / bias x8 on the host) keep the
math exact.  Inputs arrive in 8 consolidated DMAs issued from three
different engine queues (sync/scalar) so doorbell issue doesn't serialize
ahead of the first matmul; outputs leave as bf16 (upconverted on the
host) from the gpsimd queue.  The org matmul streams as two waves per
(t, n) PSUM group - wave1 [upDR, k0..k2] -> copy to SBUF, wave2 [k3..k5]
-> DVE add - so PE work overlaps the staged xT/WT arrival.

Host-side work is layout/dtype only; every FLOP of the reference runs on
device.
"""

import sys
import types

import numpy as np

import concourse.mybir as mybir
import concourse.tile as tile
from concourse import bacc
from concourse.bass_utils import run_bass_kernel_spmd

# If BASS_TRACE is set in the environment, run_bass_kernel_spmd imports
# antenv.axon_hooks, which some containers lack; stub it (None hook ->
# tracing is skipped gracefully, execution unaffected).
try:
    import antenv.axon_hooks  # noqa: F401
except ImportError:
    _m = types.ModuleType("antenv.axon_hooks")
    _m.get_axon_ntff_profile_hook = lambda: None
    _m.set_axon_ntff_profile_hook = lambda h: None
    sys.modules["antenv.axon_hooks"] = _m

# Problem shapes (hardcoded per spec nn_CPELayer_ResAG_19335942766951)
N_CORES = 8
B, T, DIN, DOUT = 64, 77, 768, 1280
N_CONCEPTS, RANK = 50, 4
BPC = B // N_CORES          # batches per core = 8
TOK = BPC * T               # tokens per core = 616
NJ = N_CONCEPTS * RANK      # 200 flattened (concept, rank) rows
P = 128
KD = DIN // P               # 6 k-tiles of the d_in contraction
NH = 308                    # half of TOK for the A.T psum tiles
# led layout: [:, 0:200] = ldT*16, the two 308-col halves of edT at
# 16B-aligned offsets 208 and 528 (DoubleRow requires 16-aligned bases and
# subtile steps; LED_W=848 keeps the subtile byte-step 16-aligned too).
ED_OFFS = (208, 528)
LED_W = 848
XW_W = TOK + DOUT           # 1896: [:, 0:616]=xT k-tile, [:, 616:]=WT k-tile
T_EDGES = [0, 128, 256, 384, 512, 616]
N_CHUNKS = [(0, 512), (512, 512), (1024, 256)]

_cache = {}


def _build_bass():
    nc = bacc.Bacc("TRN2", target_bir_lowering=False, debug=False,
                   num_devices=N_CORES)
    f32 = mybir.dt.float32
    bf16 = mybir.dt.bfloat16
    f8 = mybir.dt.float8e4
    DR = mybir.MatmulPerfMode.DoubleRow

    # All big inputs are partition-major [128, W] in DRAM (one fat row per
    # partition -> one large DMA descriptor per partition line, so the DMA
    # rings run at full rate instead of descriptor-rate-bound).
    led_d = nc.dram_tensor("led", [P, KD * LED_W], f8,
                           kind="ExternalInput").ap()
    lu_d = nc.dram_tensor("lu8", [P, 2 * DOUT], f8, kind="ExternalInput").ap()
    idx_d = nc.dram_tensor("idxf", [1, TOK], f32, kind="ExternalInput").ap()
    cv_d = nc.dram_tensor("cvals", [P, 2], f32, kind="ExternalInput").ap()
    xw_d = nc.dram_tensor("xw", [P, KD * XW_W], bf16,
                          kind="ExternalInput").ap()
    out_d = nc.dram_tensor("out", [TOK, DOUT], bf16, kind="ExternalOutput").ap()

    with tile.TileContext(nc) as tc:
        with (
            tc.tile_pool(name="consts", bufs=1) as consts,
            tc.tile_pool(name="outsb", bufs=5) as outsb,
        ):
            # Input DMAs: all doorbells on the sync queue in strict
            # arrival-priority order (DMA-ring work order = enqueue order,
            # so the critical LoRA-chain data cannot be starved by the big
            # xw stream).
            led_a = consts.tile([P, 2, LED_W], f8, tag="led_a")
            nc.sync.dma_start(led_a[:],
                              led_d[:, 0:2 * LED_W].rearrange(
                                  "p (k c) -> p k c", c=LED_W))
            led_b = consts.tile([P, KD - 2, LED_W], f8, tag="led_b")
            nc.sync.dma_start(led_b[:],
                              led_d[:, 2 * LED_W:KD * LED_W].rearrange(
                                  "p (k c) -> p k c", c=LED_W))
            lu8 = consts.tile([P, 2, DOUT], f8, tag="lu8")
            nc.sync.dma_start(lu8[:],
                              lu_d.rearrange("p (j o) -> p j o", o=DOUT))
            cvals = consts.tile([P, 2], f32, tag="cvals")
            nc.sync.dma_start(cvals[:], cv_d[:, :])
            idx_row = consts.tile([1, TOK], f32, tag="idx_row")
            nc.sync.dma_start(idx_row[:], idx_d[:, :])
            xw0 = consts.tile([P, 1, XW_W], bf16, tag="xw0")
            nc.sync.dma_start(xw0[:],
                              xw_d[:, 0:XW_W].rearrange(
                                  "p (k c) -> p k c", c=XW_W))
            xw12 = consts.tile([P, 2, XW_W], bf16, tag="xw12")
            nc.sync.dma_start(xw12[:],
                              xw_d[:, XW_W:3 * XW_W].rearrange(
                                  "p (k c) -> p k c", c=XW_W))
            xw345 = consts.tile([P, 3, XW_W], bf16, tag="xw345")
            nc.sync.dma_start(xw345[:],
                              xw_d[:, 3 * XW_W:KD * XW_W].rearrange(
                                  "p (k c) -> p k c", c=XW_W))

            # Token->concept ids broadcast across partitions on-device
            # (a 2.5KB DMA + one gpsimd op instead of 315KB of ring work).
            idx_bc = consts.tile([P, TOK], f32, tag="idx_bc")
            nc.gpsimd.partition_broadcast(idx_bc[:], idx_row[:])

            def led_pair(kk, csl):  # 3D [128, 2, csl] DoubleRow operand
                t_ = led_a if kk < 2 else led_b
                o = 0 if kk < 2 else 2
                return t_[:, kk - o:kk - o + 2, csl]

            def xt(kk, tsl):
                t_, o = ((xw0, 0) if kk < 1 else
                         (xw12, 1) if kk < 3 else (xw345, 3))
                return t_[:, kk - o, tsl]

            def wt(kk, nsl):
                t_, o = ((xw0, 0) if kk < 1 else
                         (xw12, 1) if kk < 3 else (xw345, 3))
                return t_[:, kk - o, slice(TOK + nsl.start, TOK + nsl.stop)]

            # One-hot masks: mask[p, t] = (concept_idx[t] == cvals[p, jc])
            # scaled by 2^-7 (folds away the host-side ld x16 and the fp8
            # dynamic-range shift s=8 on MT8).
            masks = []
            for jc in range(2):
                m = consts.tile([P, TOK], f32, tag=f"mask{jc}")
                nc.vector.tensor_scalar(
                    m[:], idx_bc[:], cvals[:, jc:jc + 1], 1.0 / 128.0,
                    mybir.AluOpType.is_equal, mybir.AluOpType.mult)
                masks.append(m)

            # MT8[(c,r) rows as [part, subtile], t]: routed "down" activations
            # in fp8, consumed by the DoubleRow up-matmul.  Rows 200..255
            # (partitions 72..127 of subtile 1) are zero except the ones row
            # at 224 (partition 96) that contracts with the bias row of lu8.
            MT8 = consts.tile([P, 2, 640], f8, tag="MT8")
            nc.gpsimd.memset(MT8[64:P, 1, :], 0.0)
            nc.gpsimd.memset(MT8[96:97, 1, :], 0.125)

            # A.T[(c,r), t] = (16*lora_down_flat).T @ ed.T for all concepts,
            # 3 DoubleRow matmuls per chunk (contraction row = sub*128+p).
            with tc.tile_pool(name="at_ps", bufs=4, space="PSUM") as at_pool:
                for jc in range(2):
                    jp = P if jc == 0 else NJ - P  # 128, 72
                    jsl = slice(jc * P, jc * P + jp)
                    for nh in range(2):
                        nsl = slice(nh * NH, (nh + 1) * NH)
                        esl = slice(ED_OFFS[nh], ED_OFFS[nh] + NH)
                        at = at_pool.tile([P, NH], f32, tag="at")
                        for k in range(KD // 2):
                            nc.tensor.matmul(
                                at[:jp, :], led_pair(2 * k, jsl),
                                led_pair(2 * k, esl),
                                start=(k == 0), stop=(k == KD // 2 - 1),
                                perf_mode=DR)
                        nc.vector.tensor_tensor(
                            MT8[:jp, jc, nsl], at[:jp, :],
                            masks[jc][:jp, nsl], mybir.AluOpType.mult)

            # Main accumulation: wave1 [upDR, org k0..k2] per (t, n) PSUM
            # group -> copy to bf16 osb (alternating scalar/vector); wave2
            # [org k3..k5] -> vector add into osb -> bf16 output DMA from
            # the gpsimd queue.
            KA = 3
            with tc.tile_pool(name="out_ps", bufs=8, space="PSUM") as out_pool:
                osbs = []
                ei = 0
                for ti in range(len(T_EDGES) - 1):
                    t0, t1 = T_EDGES[ti], T_EDGES[ti + 1]
                    tw = t1 - t0
                    tsl = slice(t0, t1)
                    osb = outsb.tile([P, DOUT], bf16, tag="osb")
                    osbs.append(osb)
                    for (n0, nw) in N_CHUNKS:
                        nsl = slice(n0, n0 + nw)
                        ps = out_pool.tile([P, 512], f32, tag="ops")
                        nc.tensor.matmul(
                            ps[:tw, :nw], MT8[:, 0:2, tsl],
                            lu8[:, 0:2, nsl],
                            start=True, stop=False, perf_mode=DR)
                        for k in range(KA):
                            nc.tensor.matmul(
                                ps[:tw, :nw], xt(k, tsl), wt(k, nsl),
                                start=False, stop=(k == KA - 1))
                        if ei % 2 == 0:
                            nc.scalar.copy(out=osb[:tw, nsl],
                                           in_=ps[:tw, :nw])
                        else:
                            nc.vector.tensor_copy(out=osb[:tw, nsl],
                                                  in_=ps[:tw, :nw])
                        ei += 1
                for ti in range(len(T_EDGES) - 1):
                    t0, t1 = T_EDGES[ti], T_EDGES[ti + 1]
                    tw = t1 - t0
                    tsl = slice(t0, t1)
                    osb = osbs[ti]
                    for (n0, nw) in N_CHUNKS:
                        nsl = slice(n0, n0 + nw)
                        ps = out_pool.tile([P, 512], f32, tag="ops")
                        for i, k in enumerate(range(KA, KD)):
                            nc.tensor.matmul(
                                ps[:tw, :nw], xt(k, tsl), wt(k, nsl),
                                start=(i == 0), stop=(i == KD - KA - 1))
                        nc.vector.tensor_tensor(
                            osb[:tw, nsl], ps[:tw, :nw], osb[:tw, nsl],
                            mybir.AluOpType.add)
                        if ti == len(T_EDGES) - 2:
                            # last t-tile: ship each n-chunk as soon as its
                            # add lands, so the final drain is one small DMA
                            nc.gpsimd.dma_start(out_d[tsl, nsl],
                                                osb[:tw, nsl])
                    if ti != len(T_EDGES) - 2:
                        nc.gpsimd.dma_start(out_d[tsl, :], osb[:tw, :])

    nc.compile()
    return nc


def get_bass():
    if "v2" not in _cache:
        _cache["v2"] = _build_bass()
    return _cache["v2"]


def make_in_maps(x, edit_direction, concept_idx, lora_down, lora_up, W, b_lin):
    """Host-side sharding + layout/dtype prep (no reference FLOPs)."""
    f8 = mybir.dt.np(mybir.dt.float8e4)
    bf = mybir.dt.np(mybir.dt.bfloat16)
    x = np.asarray(x, dtype=np.float32)
    ed = np.asarray(edit_direction, dtype=np.float32)
    idx = np.asarray(concept_idx)
    ld = np.asarray(lora_down, dtype=np.float32)
    lup = np.asarray(lora_up, dtype=np.float32)
    W = np.asarray(W, dtype=np.float32)
    b = np.asarray(b_lin, dtype=np.float32)

    def pmajor(a, ksub):
        """[ksub*128, W] row-major -> [128, ksub*W] partition-major."""
        w = a.shape[1]
        return np.ascontiguousarray(
            a.reshape(ksub, P, w).transpose(1, 0, 2).reshape(P, ksub * w))

    ldT = ld.transpose(1, 0, 2).reshape(DIN, NJ)                # [768, 200]
    lu8 = np.zeros((2 * P, DOUT), dtype=np.float32)
    lu8[:NJ] = lup.reshape(NJ, DOUT) * 2.0   # x8 (range) x0.25 (alpha/rank)
    lu8[P + 96] = b * 8.0                    # bias row (ones row is 1/8)
    lu8 = pmajor(lu8.astype(f8), 2)
    cv = np.full(2 * P, -1.0, dtype=np.float32)
    cv[:NJ] = np.arange(NJ, dtype=np.float32) // RANK
    cvals = np.ascontiguousarray(cv.reshape(2, P).T)            # [128, 2]
    WT = W.T.astype(bf)                                         # [768, 1280]

    in_maps = []
    for c in range(N_CORES):
        sl = slice(c * BPC, (c + 1) * BPC)
        xs = x[sl].reshape(TOK, DIN)
        eds = ed[sl].reshape(TOK, DIN)
        idxs = idx[sl].reshape(TOK).astype(np.float32)
        led = np.zeros((DIN, LED_W), dtype=f8)
        led[:, :NJ] = (ldT * 16.0).astype(f8)
        edT8 = eds.T.astype(f8)
        led[:, ED_OFFS[0]:ED_OFFS[0] + NH] = edT8[:, :NH]
        led[:, ED_OFFS[1]:ED_OFFS[1] + NH] = edT8[:, NH:]
        xw = np.empty((DIN, XW_W), dtype=bf)
        xw[:, :TOK] = xs.T.astype(bf)
        xw[:, TOK:] = WT
        in_maps.append({
            "led": pmajor(led, KD),
            "lu8": lu8,
            "idxf": np.ascontiguousarray(idxs.reshape(1, TOK)),
            "cvals": cvals,
            "xw": pmajor(xw, KD),
        })
    return in_maps


def kernel(x, edit_direction, concept_idx, lora_down, lora_up, W, b_lin,
           _trace=False):
    nc = get_bass()
    in_maps = make_in_maps(x, edit_direction, concept_idx, lora_down, lora_up,
                           W, b_lin)
    res = run_bass_kernel_spmd(nc, in_maps, core_ids=list(range(N_CORES)),
                               trace=_trace)
    out = np.concatenate([np.asarray(r["out"], dtype=np.float32)
                          for r in res.results], axis=0)
    out = out.reshape(B, T, DOUT)
    if _trace:
        kernel.last_results = res
    return out


# revision 19
# speedup vs baseline: 1.2500x; 1.2500x over previous
"""Trainium2 Bass kernel for CPELayer_ResAG (concept-routed LoRA edit layer).

Computation (per token t with concept c = concept_idx[t]):
    down = edit_direction[t] @ lora_down[c]          # [768]@[768,4] -> [4]
    up   = down @ lora_up[c]                         # [4]@[4,1280]  -> [1280]
    out  = x[t] @ W.T + b_lin + 0.25 * up

Strategy: data-parallel over batch across 8 cores (616 tokens/core).
The routed LoRA is computed densely for ALL concepts (A.T = ld.T @ ed.T,
~6% extra PE work), masked on-device with a one-hot built by DVE is_equal,
and contracted back with lora_up, accumulating into the same PSUM group as
the org matmul.

v2 schedule (vs v1): LoRA-branch matmuls run in fp8-e4m3 with
perf_mode=DoubleRow (2 contraction rows per PE cell: operands are 3D
[128, 2, N] tiles, contraction row = subtile*128 + partition), halving
their PE column counts.  All power-of-two scale corrections (ld x16 on
the host, x1/128 in the mask value, lu x2 / bias x8 on the host) keep the
math exact.  Inputs arrive in 8 consolidated DMAs issued from three
different engine queues (sync/scalar) so doorbell issue doesn't serialize
ahead of the first matmul; outputs leave as bf16 (upconverted on the
host) from the gpsimd queue.  The org matmul streams as two waves per
(t, n) PSUM group - wave1 [upDR, k0..k2] -> copy to SBUF, wave2 [k3..k5]
-> DVE add - so PE work overlaps the staged xT/WT arrival.

Host-side work is layout/dtype only; every FLOP of the reference runs on
device.
"""

import sys
import types

import numpy as np

import concourse.mybir as mybir
import concourse.tile as tile
from concourse import bacc
from concourse.bass_utils import run_bass_kernel_spmd

# If BASS_TRACE is set in the environment, run_bass_kernel_spmd imports
# antenv.axon_hooks, which some containers lack; stub it (None hook ->
# tracing is skipped gracefully, execution unaffected).
try:
    import antenv.axon_hooks  # noqa: F401
except ImportError:
    _m = types.ModuleType("antenv.axon_hooks")
    _m.get_axon_ntff_profile_hook = lambda: None
    _m.set_axon_ntff_profile_hook = lambda h: None
    sys.modules["antenv.axon_hooks"] = _m

# Problem shapes (hardcoded per spec nn_CPELayer_ResAG_19335942766951)
N_CORES = 8
B, T, DIN, DOUT = 64, 77, 768, 1280
N_CONCEPTS, RANK = 50, 4
BPC = B // N_CORES          # batches per core = 8
TOK = BPC * T               # tokens per core = 616
NJ = N_CONCEPTS * RANK      # 200 flattened (concept, rank) rows
P = 128
KD = DIN // P               # 6 k-tiles of the d_in contraction
NH = 308                    # half of TOK for the A.T psum tiles
# led layout, split by ed-half so the first two A.T chains (nh=0) only
# need the first DMA: block A [128, 6, 528] holds ldT*16 at cols 0:200 and
# ed half0 at cols 208:516; block B [128, 6, 336] holds ed half1 at 0:308.
# All DoubleRow bases and subtile byte-steps are 16-aligned (208/528/336).
ED_OFF = 208
LEDA_W = 528
LEDB_W = 336
XW_W = TOK + DOUT           # 1896: [:, 0:616]=xT k-tile, [:, 616:]=WT k-tile
T_EDGES = [0, 128, 256, 384, 512, 616]
N_CHUNKS = [(0, 512), (512, 512), (1024, 256)]

_cache = {}


def _build_bass():
    nc = bacc.Bacc("TRN2", target_bir_lowering=False, debug=False,
                   num_devices=N_CORES)
    f32 = mybir.dt.float32
    bf16 = mybir.dt.bfloat16
    f8 = mybir.dt.float8e4
    DR = mybir.MatmulPerfMode.DoubleRow

    # All big inputs are partition-major [128, W] in DRAM (one fat row per
    # partition -> one large DMA descriptor per partition line, so the DMA
    # rings run at full rate instead of descriptor-rate-bound).
    led_d = nc.dram_tensor("led", [P, KD * (LEDA_W + LEDB_W)], f8,
                           kind="ExternalInput").ap()
    lu_d = nc.dram_tensor("lu8", [P, 2 * DOUT], f8, kind="ExternalInput").ap()
    idx_d = nc.dram_tensor("idxf", [1, TOK], f32, kind="ExternalInput").ap()
    cv_d = nc.dram_tensor("cvals", [P, 2], f32, kind="ExternalInput").ap()
    xw_d = nc.dram_tensor("xw", [P, KD * XW_W], bf16,
                          kind="ExternalInput").ap()
    out_d = nc.dram_tensor("out", [TOK, DOUT], bf16, kind="ExternalOutput").ap()

    with tile.TileContext(nc) as tc:
        with (
            tc.tile_pool(name="consts", bufs=1) as consts,
            tc.tile_pool(name="outsb", bufs=5) as outsb,
        ):
            # Input DMAs: all doorbells on the sync queue in strict
            # arrival-priority order (DMA-ring work order = enqueue order,
            # so the critical LoRA-chain data cannot be starved by the big
            # xw stream).
            led_a = consts.tile([P, KD, LEDA_W], f8, tag="led_a")
            nc.sync.dma_start(led_a[:],
                              led_d[:, 0:KD * LEDA_W].rearrange(
                                  "p (k c) -> p k c", c=LEDA_W))
            led_b = consts.tile([P, KD, LEDB_W], f8, tag="led_b")
            nc.sync.dma_start(led_b[:],
                              led_d[:, KD * LEDA_W:].rearrange(
                                  "p (k c) -> p k c", c=LEDB_W))
            lu8 = consts.tile([P, 2, DOUT], f8, tag="lu8")
            nc.sync.dma_start(lu8[:],
                              lu_d.rearrange("p (j o) -> p j o", o=DOUT))
            cvals = consts.tile([P, 2], f32, tag="cvals")
            nc.sync.dma_start(cvals[:], cv_d[:, :])
            idx_bc = consts.tile([P, TOK], f32, tag="idx_bc")
            nc.sync.dma_start(idx_bc[:], idx_d.partition_broadcast(P))
            xw0 = consts.tile([P, 1, XW_W], bf16, tag="xw0")
            nc.sync.dma_start(xw0[:],
                              xw_d[:, 0:XW_W].rearrange(
                                  "p (k c) -> p k c", c=XW_W))
            xw12 = consts.tile([P, 2, XW_W], bf16, tag="xw12")
            nc.sync.dma_start(xw12[:],
                              xw_d[:, XW_W:3 * XW_W].rearrange(
                                  "p (k c) -> p k c", c=XW_W))
            xw345 = consts.tile([P, 3, XW_W], bf16, tag="xw345")
            nc.sync.dma_start(xw345[:],
                              xw_d[:, 3 * XW_W:KD * XW_W].rearrange(
                                  "p (k c) -> p k c", c=XW_W))

            # Hoist the scalar engine's one-time ACT_TABLE_LOAD into the
            # boot shadow (it otherwise fires lazily right before the first
            # wave1 copy, delaying it by ~1.5us).
            scratch = consts.tile([1, 8], f32, tag="scratch")
            nc.vector.memset(scratch[:], 0.0)
            nc.scalar.copy(out=scratch[:, 0:4], in_=scratch[:, 4:8])

            def xt(kk, tsl):
                t_, o = ((xw0, 0) if kk < 1 else
                         (xw12, 1) if kk < 3 else (xw345, 3))
                return t_[:, kk - o, tsl]

            def wt(kk, nsl):
                t_, o = ((xw0, 0) if kk < 1 else
                         (xw12, 1) if kk < 3 else (xw345, 3))
                return t_[:, kk - o, slice(TOK + nsl.start, TOK + nsl.stop)]

            # One-hot masks: mask[p, t] = (concept_idx[t] == cvals[p, jc])
            # scaled by 2^-7 (folds away the host-side ld x16 and the fp8
            # dynamic-range shift s=8 on MT8).
            masks = []
            for jc in range(2):
                m = consts.tile([P, TOK], f32, tag=f"mask{jc}")
                nc.vector.tensor_scalar(
                    m[:], idx_bc[:], cvals[:, jc:jc + 1], 1.0 / 128.0,
                    mybir.AluOpType.is_equal, mybir.AluOpType.mult)
                masks.append(m)

            # MT8[(c,r) rows as [part, subtile], t]: routed "down" activations
            # in fp8, consumed by the DoubleRow up-matmul.  Rows 200..255
            # (partitions 72..127 of subtile 1) are zero except the ones row
            # at 224 (partition 96) that contracts with the bias row of lu8.
            MT8 = consts.tile([P, 2, 640], f8, tag="MT8")
            nc.gpsimd.memset(MT8[64:P, 1, :], 0.0)
            nc.gpsimd.memset(MT8[96:97, 1, :], 0.125)

            # A.T[(c,r), t] = (16*lora_down_flat).T @ ed.T for all concepts,
            # 3 DoubleRow matmuls per chunk (contraction row = sub*128+p).
            # nh=0 chains read only led_a, so they start before led_b lands.
            with tc.tile_pool(name="at_ps", bufs=4, space="PSUM") as at_pool:
                for nh in range(2):
                    for jc in range(2):
                        jp = P if jc == 0 else NJ - P  # 128, 72
                        jsl = slice(jc * P, jc * P + jp)
                        nsl = slice(nh * NH, (nh + 1) * NH)
                        at = at_pool.tile([P, NH], f32, tag="at")
                        for k in range(KD // 2):
                            if nh == 0:
                                rhs = led_a[:, 2 * k:2 * k + 2,
                                            ED_OFF:ED_OFF + NH]
                            else:
                                rhs = led_b[:, 2 * k:2 * k + 2, 0:NH]
                            nc.tensor.matmul(
                                at[:jp, :],
                                led_a[:, 2 * k:2 * k + 2, jsl], rhs,
                                start=(k == 0), stop=(k == KD // 2 - 1),
                                perf_mode=DR)
                        nc.vector.tensor_tensor(
                            MT8[:jp, jc, nsl], at[:jp, :],
                            masks[jc][:jp, nsl], mybir.AluOpType.mult)

            # Main accumulation: wave1 [upDR, org k0..k2] per (t, n) PSUM
            # group -> copy to bf16 osb (alternating scalar/vector); wave2
            # [org k3..k5] -> vector add into osb -> bf16 output DMA from
            # the gpsimd queue.
            KA = 3
            with tc.tile_pool(name="out_ps", bufs=8, space="PSUM") as out_pool:
                osbs = []
                ei = 0
                for ti in range(len(T_EDGES) - 1):
                    t0, t1 = T_EDGES[ti], T_EDGES[ti + 1]
                    tw = t1 - t0
                    tsl = slice(t0, t1)
                    osb = outsb.tile([P, DOUT], bf16, tag="osb")
                    osbs.append(osb)
                    for (n0, nw) in N_CHUNKS:
                        nsl = slice(n0, n0 + nw)
                        ps = out_pool.tile([P, 512], f32, tag="ops")
                        nc.tensor.matmul(
                            ps[:tw, :nw], MT8[:, 0:2, tsl],
                            lu8[:, 0:2, nsl],
                            start=True, stop=False, perf_mode=DR)
                        for k in range(KA):
                            nc.tensor.matmul(
                                ps[:tw, :nw], xt(k, tsl), wt(k, nsl),
                                start=False, stop=(k == KA - 1))
                        if ei % 2 == 0:
                            nc.scalar.copy(out=osb[:tw, nsl],
                                           in_=ps[:tw, :nw])
                        else:
                            nc.vector.tensor_copy(out=osb[:tw, nsl],
                                                  in_=ps[:tw, :nw])
                        ei += 1
                for ti in range(len(T_EDGES) - 1):
                    t0, t1 = T_EDGES[ti], T_EDGES[ti + 1]
                    tw = t1 - t0
                    tsl = slice(t0, t1)
                    osb = osbs[ti]
                    for (n0, nw) in N_CHUNKS:
                        nsl = slice(n0, n0 + nw)
                        ps = out_pool.tile([P, 512], f32, tag="ops")
                        for i, k in enumerate(range(KA, KD)):
                            nc.tensor.matmul(
                                ps[:tw, :nw], xt(k, tsl), wt(k, nsl),
                                start=(i == 0), stop=(i == KD - KA - 1))
                        nc.vector.tensor_tensor(
                            osb[:tw, nsl], ps[:tw, :nw], osb[:tw, nsl],
                            mybir.AluOpType.add)
                        if ti == len(T_EDGES) - 2:
                            # last t-tile: ship each n-chunk as soon as its
                            # add lands, so the final drain is one small DMA
                            nc.gpsimd.dma_start(out_d[tsl, nsl],
                                                osb[:tw, nsl])
                    if ti != len(T_EDGES) - 2:
                        nc.gpsimd.dma_start(out_d[tsl, :], osb[:tw, :])

    nc.compile()
    return nc


def get_bass():
    if "v2" not in _cache:
        _cache["v2"] = _build_bass()
    return _cache["v2"]


def make_in_maps(x, edit_direction, concept_idx, lora_down, lora_up, W, b_lin):
    """Host-side sharding + layout/dtype prep (no reference FLOPs)."""
    f8 = mybir.dt.np(mybir.dt.float8e4)
    bf = mybir.dt.np(mybir.dt.bfloat16)
    x = np.asarray(x, dtype=np.float32)
    ed = np.asarray(edit_direction, dtype=np.float32)
    idx = np.asarray(concept_idx)
    ld = np.asarray(lora_down, dtype=np.float32)
    lup = np.asarray(lora_up, dtype=np.float32)
    W = np.asarray(W, dtype=np.float32)
    b = np.asarray(b_lin, dtype=np.float32)

    def pmajor(a, ksub):
        """[ksub*128, W] row-major -> [128, ksub*W] partition-major."""
        w = a.shape[1]
        return np.ascontiguousarray(
            a.reshape(ksub, P, w).transpose(1, 0, 2).reshape(P, ksub * w))

    ldT = ld.transpose(1, 0, 2).reshape(DIN, NJ)                # [768, 200]
    lu8 = np.zeros((2 * P, DOUT), dtype=np.float32)
    lu8[:NJ] = lup.reshape(NJ, DOUT) * 2.0   # x8 (range) x0.25 (alpha/rank)
    lu8[P + 96] = b * 8.0                    # bias row (ones row is 1/8)
    lu8 = pmajor(lu8.astype(f8), 2)
    cv = np.full(2 * P, -1.0, dtype=np.float32)
    cv[:NJ] = np.arange(NJ, dtype=np.float32) // RANK
    cvals = np.ascontiguousarray(cv.reshape(2, P).T)            # [128, 2]
    WT = W.T.astype(bf)                                         # [768, 1280]

    in_maps = []
    for c in range(N_CORES):
        sl = slice(c * BPC, (c + 1) * BPC)
        xs = x[sl].reshape(TOK, DIN)
        eds = ed[sl].reshape(TOK, DIN)
        idxs = idx[sl].reshape(TOK).astype(np.float32)
        leda = np.zeros((DIN, LEDA_W), dtype=f8)
        leda[:, :NJ] = (ldT * 16.0).astype(f8)
        edT8 = eds.T.astype(f8)
        leda[:, ED_OFF:ED_OFF + NH] = edT8[:, :NH]
        ledb = np.zeros((DIN, LEDB_W), dtype=f8)
        ledb[:, :NH] = edT8[:, NH:]
        led = np.concatenate([pmajor(leda, KD), pmajor(ledb, KD)], axis=1)
        xw = np.empty((DIN, XW_W), dtype=bf)
        xw[:, :TOK] = xs.T.astype(bf)
        xw[:, TOK:] = WT
        in_maps.append({
            "led": np.ascontiguousarray(led),
            "lu8": lu8,
            "idxf": np.ascontiguousarray(idxs.reshape(1, TOK)),
            "cvals": cvals,
            "xw": pmajor(xw, KD),
        })
    return in_maps


def kernel(x, edit_direction, concept_idx, lora_down, lora_up, W, b_lin,
           _trace=False):
    nc = get_bass()
    in_maps = make_in_maps(x, edit_direction, concept_idx, lora_down, lora_up,
                           W, b_lin)
    res = run_bass_kernel_spmd(nc, in_maps, core_ids=list(range(N_CORES)),
                               trace=_trace)
    out = np.concatenate([np.asarray(r["out"], dtype=np.float32)
                          for r in res.results], axis=0)
    out = out.reshape(B, T, DOUT)
    if _trace:
        kernel.last_results = res
    return out


# revision 25
# speedup vs baseline: 1.3098x; 1.0479x over previous
"""Trainium2 Bass kernel for CPELayer_ResAG (concept-routed LoRA edit layer).

Computation (per token t with concept c = concept_idx[t]):
    down = edit_direction[t] @ lora_down[c]          # [768]@[768,4] -> [4]
    up   = down @ lora_up[c]                         # [4]@[4,1280]  -> [1280]
    out  = x[t] @ W.T + b_lin + 0.25 * up

Strategy: data-parallel over batch across 8 cores (616 tokens/core).
The routed LoRA is computed densely for ALL concepts (A.T = ld.T @ ed.T,
~6% extra PE work), masked on-device with a one-hot built by DVE is_equal,
and contracted back with lora_up, accumulating into the same PSUM group as
the org matmul.

v2 schedule (vs v1): LoRA-branch matmuls run in fp8-e4m3 with
perf_mode=DoubleRow (2 contraction rows per PE cell: operands are 3D
[128, 2, N] tiles, contraction row = subtile*128 + partition), halving
their PE column counts.  All power-of-two scale corrections (ld x16 on
the host, x1/128 in the mask value, lu x2 / bias x8 on the host) keep the
math exact.  Inputs arrive in 8 consolidated DMAs issued from three
different engine queues (sync/scalar) so doorbell issue doesn't serialize
ahead of the first matmul; outputs leave as bf16 (upconverted on the
host) from the gpsimd queue.  The org matmul streams as two waves per
(t, n) PSUM group - wave1 [upDR, k0..k2] -> copy to SBUF, wave2 [k3..k5]
-> DVE add - so PE work overlaps the staged xT/WT arrival.

Host-side work is layout/dtype only; every FLOP of the reference runs on
device.
"""

import sys
import types

import numpy as np

import concourse.mybir as mybir
import concourse.tile as tile
from concourse import bacc
from concourse.bass_utils import run_bass_kernel_spmd

# If BASS_TRACE is set in the environment, run_bass_kernel_spmd imports
# antenv.axon_hooks, which some containers lack; stub it (None hook ->
# tracing is skipped gracefully, execution unaffected).
try:
    import antenv.axon_hooks  # noqa: F401
except ImportError:
    _m = types.ModuleType("antenv.axon_hooks")
    _m.get_axon_ntff_profile_hook = lambda: None
    _m.set_axon_ntff_profile_hook = lambda h: None
    sys.modules["antenv.axon_hooks"] = _m

# Problem shapes (hardcoded per spec nn_CPELayer_ResAG_19335942766951)
N_CORES = 8
B, T, DIN, DOUT = 64, 77, 768, 1280
N_CONCEPTS, RANK = 50, 4
BPC = B // N_CORES          # batches per core = 8
TOK = BPC * T               # tokens per core = 616
NJ = N_CONCEPTS * RANK      # 200 flattened (concept, rank) rows
P = 128
KD = DIN // P               # 6 k-tiles of the d_in contraction
NH = 308                    # half of TOK for the A.T psum tiles
# led layout, split by ed-half so the first two A.T chains (nh=0) only
# need the first DMA: block A [128, 6, 528] holds ldT*16 at cols 0:200 and
# ed half0 at cols 208:516; block B [128, 6, 336] holds ed half1 at 0:308.
# All DoubleRow bases and subtile byte-steps are 16-aligned (208/528/336).
ED_OFF = 208
LEDA_W = 528
LEDB_W = 336
XW_W = TOK + DOUT           # 1896: [:, 0:616]=xT k-tile, [:, 616:]=WT k-tile
T_EDGES = [0, 128, 256, 384, 512, 616]
N_CHUNKS = [(0, 512), (512, 512), (1024, 256)]

_cache = {}


def _build_bass():
    nc = bacc.Bacc("TRN2", target_bir_lowering=False, debug=False,
                   num_devices=N_CORES)
    f32 = mybir.dt.float32
    bf16 = mybir.dt.bfloat16
    f8 = mybir.dt.float8e4
    DR = mybir.MatmulPerfMode.DoubleRow

    # All big inputs are partition-major [128, W] in DRAM (one fat row per
    # partition -> one large DMA descriptor per partition line, so the DMA
    # rings run at full rate instead of descriptor-rate-bound).
    led_d = nc.dram_tensor("led", [P, KD * (LEDA_W + LEDB_W)], f8,
                           kind="ExternalInput").ap()
    lu_d = nc.dram_tensor("lu8", [P, 2 * DOUT], f8, kind="ExternalInput").ap()
    idx_d = nc.dram_tensor("idxf", [P, TOK], f32, kind="ExternalInput").ap()
    cv_d = nc.dram_tensor("cvals", [P, 2], f32, kind="ExternalInput").ap()
    xw_d = nc.dram_tensor("xw", [P, KD * XW_W], bf16,
                          kind="ExternalInput").ap()
    out_d = nc.dram_tensor("out", [TOK, DOUT], bf16, kind="ExternalOutput").ap()

    with tile.TileContext(nc) as tc:
        with (
            tc.tile_pool(name="consts", bufs=1) as consts,
            tc.tile_pool(name="outsb", bufs=5) as outsb,
        ):
            # Input DMAs: all doorbells on the sync queue in strict
            # arrival-priority order (DMA-ring work order = enqueue order,
            # so the critical LoRA-chain data cannot be starved by the big
            # xw stream).
            led_a = consts.tile([P, KD, LEDA_W], f8, tag="led_a")
            nc.sync.dma_start(led_a[:],
                              led_d[:, 0:KD * LEDA_W].rearrange(
                                  "p (k c) -> p k c", c=LEDA_W))
            led_b = consts.tile([P, KD, LEDB_W], f8, tag="led_b")
            nc.sync.dma_start(led_b[:],
                              led_d[:, KD * LEDA_W:].rearrange(
                                  "p (k c) -> p k c", c=LEDB_W))
            lu8 = consts.tile([P, 2, DOUT], f8, tag="lu8")
            nc.sync.dma_start(lu8[:],
                              lu_d.rearrange("p (j o) -> p j o", o=DOUT))
            cvals = consts.tile([P, 2], f32, tag="cvals")
            nc.sync.dma_start(cvals[:], cv_d[:, :])
            # idx pre-broadcast on the host: a plain [128, TOK] DMA moves at
            # full ring rate, while a partition_broadcast DMA pattern ticks
            # its completion semaphore once per ~256ns for ~3us, gating the
            # masks (observed in every earlier variant).
            idx_bc = consts.tile([P, TOK], f32, tag="idx_bc")
            nc.sync.dma_start(idx_bc[:], idx_d[:, :])
            xw0 = consts.tile([P, 1, XW_W], bf16, tag="xw0")
            nc.sync.dma_start(xw0[:],
                              xw_d[:, 0:XW_W].rearrange(
                                  "p (k c) -> p k c", c=XW_W))
            xw12 = consts.tile([P, 2, XW_W], bf16, tag="xw12")
            nc.sync.dma_start(xw12[:],
                              xw_d[:, XW_W:3 * XW_W].rearrange(
                                  "p (k c) -> p k c", c=XW_W))
            xw345 = consts.tile([P, 3, XW_W], bf16, tag="xw345")
            nc.sync.dma_start(xw345[:],
                              xw_d[:, 3 * XW_W:KD * XW_W].rearrange(
                                  "p (k c) -> p k c", c=XW_W))

            # Hoist the scalar engine's one-time ACT_TABLE_LOAD into the
            # boot shadow (it otherwise fires lazily right before the first
            # wave1 copy, delaying it by ~1.5us).
            scratch = consts.tile([1, 8], f32, tag="scratch")
            nc.vector.memset(scratch[:], 0.0)
            nc.scalar.copy(out=scratch[:, 0:4], in_=scratch[:, 4:8])

            # PE warmup: ~3us of dummy matmuls on zeros during the boot/DMA
            # shadow so the tensor engine's clock is fully ramped (2.4GHz)
            # when the real A.T chain starts; cold-start matmuls otherwise
            # run ~2-3x slower for the first ~3us of execution.
            warm = consts.tile([P, 256], bf16, tag="warm")
            nc.vector.memset(warm[:], 0.0)
            with tc.tile_pool(name="warm_ps", bufs=1, space="PSUM") as wp:
                wps = wp.tile([P, 256], f32, tag="warm_ps")
                NWARM = 14
                for i in range(NWARM):
                    nc.tensor.matmul(wps[:, :], warm[:, 0:P], warm[:, :],
                                     start=(i == 0), stop=(i == NWARM - 1))

            def xt(kk, tsl):
                t_, o = ((xw0, 0) if kk < 1 else
                         (xw12, 1) if kk < 3 else (xw345, 3))
                return t_[:, kk - o, tsl]

            def wt(kk, nsl):
                t_, o = ((xw0, 0) if kk < 1 else
                         (xw12, 1) if kk < 3 else (xw345, 3))
                return t_[:, kk - o, slice(TOK + nsl.start, TOK + nsl.stop)]

            # One-hot masks: mask[p, t] = (concept_idx[t] == cvals[p, jc])
            # scaled by 2^-7 (folds away the host-side ld x16 and the fp8
            # dynamic-range shift s=8 on MT8).
            masks = []
            for jc in range(2):
                m = consts.tile([P, TOK], f32, tag=f"mask{jc}")
                nc.vector.tensor_scalar(
                    m[:], idx_bc[:], cvals[:, jc:jc + 1], 1.0 / 128.0,
                    mybir.AluOpType.is_equal, mybir.AluOpType.mult)
                masks.append(m)

            # MT8[(c,r) rows as [part, subtile], t]: routed "down" activations
            # in fp8, consumed by the DoubleRow up-matmul.  Rows 200..255
            # (partitions 72..127 of subtile 1) are zero except the ones row
            # at 224 (partition 96) that contracts with the bias row of lu8.
            MT8 = consts.tile([P, 2, 640], f8, tag="MT8")
            nc.gpsimd.memset(MT8[64:P, 1, :], 0.0)
            nc.gpsimd.memset(MT8[96:97, 1, :], 0.125)

            # A.T[(c,r), t] = (16*lora_down_flat).T @ ed.T for all concepts,
            # 3 DoubleRow matmuls per chunk (contraction row = sub*128+p).
            # nh=0 chains read only led_a, so they start before led_b lands.
            with tc.tile_pool(name="at_ps", bufs=4, space="PSUM") as at_pool:
                for nh in range(2):
                    for jc in range(2):
                        jp = P if jc == 0 else NJ - P  # 128, 72
                        jsl = slice(jc * P, jc * P + jp)
                        nsl = slice(nh * NH, (nh + 1) * NH)
                        at = at_pool.tile([P, NH], f32, tag="at")
                        for k in range(KD // 2):
                            if nh == 0:
                                rhs = led_a[:, 2 * k:2 * k + 2,
                                            ED_OFF:ED_OFF + NH]
                            else:
                                rhs = led_b[:, 2 * k:2 * k + 2, 0:NH]
                            nc.tensor.matmul(
                                at[:jp, :],
                                led_a[:, 2 * k:2 * k + 2, jsl], rhs,
                                start=(k == 0), stop=(k == KD // 2 - 1),
                                perf_mode=DR)
                        nc.vector.tensor_tensor(
                            MT8[:jp, jc, nsl], at[:jp, :],
                            masks[jc][:jp, nsl], mybir.AluOpType.mult)

            # Main accumulation: wave1 [upDR, org k0..k2] per (t, n) PSUM
            # group -> copy to bf16 osb (alternating scalar/vector); wave2
            # [org k3..k5] -> vector add into osb -> bf16 output DMA from
            # the gpsimd queue.
            KA = 3
            with tc.tile_pool(name="out_ps", bufs=8, space="PSUM") as out_pool:
                osbs = []
                ei = 0
                for ti in range(len(T_EDGES) - 1):
                    t0, t1 = T_EDGES[ti], T_EDGES[ti + 1]
                    tw = t1 - t0
                    tsl = slice(t0, t1)
                    osb = outsb.tile([P, DOUT], bf16, tag="osb")
                    osbs.append(osb)
                    for (n0, nw) in N_CHUNKS:
                        nsl = slice(n0, n0 + nw)
                        ps = out_pool.tile([P, 512], f32, tag="ops")
                        # DR up-matmul must open the group: a bf16->DR
                        # transition mid-group wedges the PE (hardware
                        # NRT_EXEC_UNIT_UNRECOVERABLE).
                        nc.tensor.matmul(
                            ps[:tw, :nw], MT8[:, 0:2, tsl],
                            lu8[:, 0:2, nsl],
                            start=True, stop=False, perf_mode=DR)
                        for k in range(KA):
                            nc.tensor.matmul(
                                ps[:tw, :nw], xt(k, tsl), wt(k, nsl),
                                start=False, stop=(k == KA - 1))
                        if ei % 2 == 0:
                            nc.scalar.copy(out=osb[:tw, nsl],
                                           in_=ps[:tw, :nw])
                        else:
                            nc.vector.tensor_copy(out=osb[:tw, nsl],
                                                  in_=ps[:tw, :nw])
                        ei += 1
                for ti in range(len(T_EDGES) - 1):
                    t0, t1 = T_EDGES[ti], T_EDGES[ti + 1]
                    tw = t1 - t0
                    tsl = slice(t0, t1)
                    osb = osbs[ti]
                    for (n0, nw) in N_CHUNKS:
                        nsl = slice(n0, n0 + nw)
                        ps = out_pool.tile([P, 512], f32, tag="ops")
                        for i, k in enumerate(range(KA, KD)):
                            nc.tensor.matmul(
                                ps[:tw, :nw], xt(k, tsl), wt(k, nsl),
                                start=(i == 0), stop=(i == KD - KA - 1))
                        nc.vector.tensor_tensor(
                            osb[:tw, nsl], ps[:tw, :nw], osb[:tw, nsl],
                            mybir.AluOpType.add)
                        if ti == len(T_EDGES) - 2:
                            # last t-tile: ship each n-chunk as soon as its
                            # add lands, so the final drain is one small DMA
                            nc.gpsimd.dma_start(out_d[tsl, nsl],
                                                osb[:tw, nsl])
                    if ti != len(T_EDGES) - 2:
                        nc.gpsimd.dma_start(out_d[tsl, :], osb[:tw, :])

    nc.compile()
    return nc


def get_bass():
    if "v2" not in _cache:
        _cache["v2"] = _build_bass()
    return _cache["v2"]


def make_in_maps(x, edit_direction, concept_idx, lora_down, lora_up, W, b_lin):
    """Host-side sharding + layout/dtype prep (no reference FLOPs)."""
    f8 = mybir.dt.np(mybir.dt.float8e4)
    bf = mybir.dt.np(mybir.dt.bfloat16)
    x = np.asarray(x, dtype=np.float32)
    ed = np.asarray(edit_direction, dtype=np.float32)
    idx = np.asarray(concept_idx)
    ld = np.asarray(lora_down, dtype=np.float32)
    lup = np.asarray(lora_up, dtype=np.float32)
    W = np.asarray(W, dtype=np.float32)
    b = np.asarray(b_lin, dtype=np.float32)

    def pmajor(a, ksub):
        """[ksub*128, W] row-major -> [128, ksub*W] partition-major."""
        w = a.shape[1]
        return np.ascontiguousarray(
            a.reshape(ksub, P, w).transpose(1, 0, 2).reshape(P, ksub * w))

    ldT = ld.transpose(1, 0, 2).reshape(DIN, NJ)                # [768, 200]
    lu8 = np.zeros((2 * P, DOUT), dtype=np.float32)
    lu8[:NJ] = lup.reshape(NJ, DOUT) * 2.0   # x8 (range) x0.25 (alpha/rank)
    lu8[P + 96] = b * 8.0                    # bias row (ones row is 1/8)
    lu8 = pmajor(lu8.astype(f8), 2)
    cv = np.full(2 * P, -1.0, dtype=np.float32)
    cv[:NJ] = np.arange(NJ, dtype=np.float32) // RANK
    cvals = np.ascontiguousarray(cv.reshape(2, P).T)            # [128, 2]
    WT = W.T.astype(bf)                                         # [768, 1280]

    in_maps = []
    for c in range(N_CORES):
        sl = slice(c * BPC, (c + 1) * BPC)
        xs = x[sl].reshape(TOK, DIN)
        eds = ed[sl].reshape(TOK, DIN)
        idxs = idx[sl].reshape(TOK).astype(np.float32)
        leda = np.zeros((DIN, LEDA_W), dtype=f8)
        leda[:, :NJ] = (ldT * 16.0).astype(f8)
        edT8 = eds.T.astype(f8)
        leda[:, ED_OFF:ED_OFF + NH] = edT8[:, :NH]
        ledb = np.zeros((DIN, LEDB_W), dtype=f8)
        ledb[:, :NH] = edT8[:, NH:]
        led = np.concatenate([pmajor(leda, KD), pmajor(ledb, KD)], axis=1)
        xw = np.empty((DIN, XW_W), dtype=bf)
        xw[:, :TOK] = xs.T.astype(bf)
        xw[:, TOK:] = WT
        in_maps.append({
            "led": np.ascontiguousarray(led),
            "lu8": lu8,
            "idxf": np.ascontiguousarray(
                np.broadcast_to(idxs.reshape(1, TOK), (P, TOK))),
            "cvals": cvals,
            "xw": pmajor(xw, KD),
        })
    return in_maps


def kernel(x, edit_direction, concept_idx, lora_down, lora_up, W, b_lin,
           _trace=False):
    nc = get_bass()
    in_maps = make_in_maps(x, edit_direction, concept_idx, lora_down, lora_up,
                           W, b_lin)
    res = run_bass_kernel_spmd(nc, in_maps, core_ids=list(range(N_CORES)),
                               trace=_trace)
    out = np.concatenate([np.asarray(r["out"], dtype=np.float32)
                          for r in res.results], axis=0)
    out = out.reshape(B, T, DOUT)
    if _trace:
        kernel.last_results = res
    return out
